# revision 14
# baseline (speedup 1.0000x reference)
"""Trainium2 Bass kernel for dual-branch (low-rank + full-rank) self-attention.

Math (per batch b, head h):
  q = x @ Wq_cat[h].T   (N, 224)   224 = 64 (lr) + 160 (full)
  scoresT[m, n] = sum_d K[m, d] Q[n, d]           (keys m on partitions)
  expT = exp(SCALE * scoresT)                     (no max subtraction; f32 psum)
  xav[d, n] = sum_m Vaug[m, d] expT[m, n]         Vaug has a ones column ->
                                                  row 96 of the hi psum = denom
  xnorm = xav * (1/denom)  (per-query recip broadcast via stride-0 DRAM DMA)
  outT[c, n] = sum_t wo_pk[t][:, c-chunk].T @ xav_pk[t]  + bias (activation add)

xav is stored PACKED: 8 heads x 224 dims = 1792 rows = exactly 14 partition
tiles (lo halves of all heads first: tile h = head h dims 0..127; then the
96-row hi parts back to back: row 1024 + h*96 + j). The output projection
contracts 14 full 128-row matmuls per output tile (vs 16 with per-head
128+97 padding) and the bias rides the psum->sbuf copy on the Scalar engine
(activation Identity with a per-partition bias vector).

Softmax denominators are normalized per-head right after each head's AV
matmuls (gather row 96 -> DRAM bounce -> reciprocal -> stride-0 broadcast
-> in-place muls), hidden under the next head's score/AV matmuls.

Sharding: data-parallel, 2 batches per core across 8 cores. No collectives.
All matmuls bf16 with f32 PSUM accumulation; softmax + normalization f32/bf16.
"""

import os
import sys

sys.path.insert(0, "/opt/trn_rl_repo")

import numpy as np
import ml_dtypes

import concourse.bass as bass
import concourse.mybir as mybir
import concourse.tile as tile
from concourse import bacc
from concourse.bass_utils import run_bass_kernel_spmd

# problem constants (hardcoded per spec)
B, N, C = 16, 1024, 1280
HEADS = 8
RANK = 64
DIM_HEAD = 160
DH = RANK + DIM_HEAD          # 224 concat head dim
SCALE = DIM_HEAD ** (-0.5)
NCORES = 8
BL = B // NCORES              # batches per core = 2
CK = C // 128                 # 10 contraction chunks over C
GROUPS = 2                    # head groups per core pass
HG = HEADS // GROUPS          # 4 heads per group
MC = N // 128                 # 8 key chunks
NT = N // 512                 # 2 query-column tiles
TOP = HEADS * DH // 128       # 14 packed xav partition tiles
N_WARM = 18                   # PE warm-up matmuls (HAM clock ungate)

BF16 = mybir.dt.bfloat16
F32 = mybir.dt.float32

# packed hi positions: head h's 96 hi dims start at global row 1024 + h*96
HI_POS = [(8 + (h * 96) // 128, (h * 96) % 128) for h in range(HEADS)]


def _hi_copy_pieces(h):
    """(src_row, dst_tile, dst_off, rows) pieces for the 96-row hi psum->
    packed-xav copy. Fragments respect the 32-aligned partition-window rule:
    base (0,0) may span freely, any other base is limited to 32 rows."""
    t0, off0 = HI_POS[h]
    pieces = []
    j = 0
    while j < 96:
        g = off0 + j
        t = t0 + g // 128
        off = g % 128
        if j == 0 and off == 0:
            n = 96
        else:
            n = min(32, 96 - j, 128 - off)
        pieces.append((j, t, off, n))
        j += n
    return pieces


def _hi_mul_pieces(h):
    """(tile, off, rows) pieces covering head h's hi rows for the in-place
    normalize multiply (base 0 spans freely, other bases 32-row pieces)."""
    t0, off0 = HI_POS[h]
    pieces = []
    j = 0
    while j < 96:
        g = off0 + j
        t = t0 + g // 128
        off = g % 128
        if off == 0:
            n = min(96 - j, 128)
        else:
            n = min(32, 96 - j, 128 - off)
        pieces.append((t, off, n))
        j += n
    return pieces


def build_bass():
    nc = bacc.Bacc("TRN2", target_bir_lowering=False, debug=False,
                   num_devices=NCORES)

    def din(name, shape, dt=BF16):
        return nc.dram_tensor(name, shape, dt, kind="ExternalInput").ap()

    xt_d = din("xt", [128, CK, BL * N])                 # x transposed, c-major
    wq_lo_d = din("wq_lo", [GROUPS, 128, CK, HG * 128])
    wq_hi_d = din("wq_hi", [GROUPS, 128, CK, HG * 96])
    wk_lo_d = din("wk_lo", [GROUPS, 128, CK, HG * 128])
    wk_hi_d = din("wk_hi", [GROUPS, 128, CK, HG * 96])
    wv_lo_d = din("wv_lo", [GROUPS, 128, CK, HG * 128])
    wv_hi_d = din("wv_hi", [GROUPS, 128, CK, HG * 97])  # 97th col zero (ones col)
    wo_pk_d = din("wo_pk", [TOP, 128, C])               # packed output proj
    bias_d = din("bias_col", [128, CK], F32)            # bias, c on partitions
    out_d = nc.dram_tensor("out", [BL, CK, 128, N], BF16,
                           kind="ExternalOutput").ap()

    with tile.TileContext(nc) as tc:
        with (
            tc.tile_pool(name="xtp", bufs=1) as xtp,
            tc.tile_pool(name="wp", bufs=4) as wp,
            tc.tile_pool(name="wop", bufs=TOP) as wop,
            tc.tile_pool(name="qkvp", bufs=1) as qkvp,
            tc.tile_pool(name="xavp", bufs=1) as xavp,
            tc.tile_pool(name="expp", bufs=2) as expp,
            tc.tile_pool(name="cstp", bufs=1) as cstp,
            tc.tile_pool(name="outp", bufs=2) as outp,
            tc.tile_pool(name="xunp", bufs=2) as xunp,
            tc.tile_pool(name="drp", bufs=4, space="DRAM") as drp,
            tc.tile_pool(name="psp", bufs=5, space="PSUM") as psp,
            tc.tile_pool(name="psavp", bufs=3, space="PSUM") as psavp,
        ):
            def ps_tile():
                # general-purpose matmul accumulators (scores/proj/out-proj)
                return psp.tile([128, 512], F32, tag="mm", name="ps")

            def ps_av_tile():
                # AV accumulators — separate slots so the normalize chain
                # never blocks the next head's scores
                return psavp.tile([128, 512], F32, tag="av", name="ps_av")

            # bias column (tiny, load first)
            bias_sb = cstp.tile([128, CK], F32, tag="bias")
            nc.sync.dma_start(bias_sb[:], bias_d)
            # denominator staging rows (2 slots: one head in flight + 1)
            den2 = cstp.tile([1, 2, N], BF16, tag="denom")

            # PE warm-up: the HAM clock gate needs ~3.4us of sustained
            # activity to ungate 2.4GHz; run zero matmuls while the first
            # xt/weight chunks stream in so real matmuls start warm.
            warm_sb = cstp.tile([128, 640], BF16, tag="warm")
            nc.gpsimd.memset(warm_sb[:], 0.0)
            ps_w = ps_tile()
            for _ in range(N_WARM):
                nc.tensor.matmul(ps_w[:], warm_sb[:, 0:128], warm_sb[:, 128:640],
                                 start=True, stop=True)

            # persistent packed output-projection weights (loaded once)
            wo_t = [wop.tile([128, C], BF16, tag="wo", name="wo")
                    for _ in range(TOP)]
            wo_loaded = [False]
            # out-proj tile order: t7/t13 hold head 7 data (normalized last),
            # so push them late in the accumulation
            T_ORDER = [0, 1, 2, 3, 4, 5, 6, 8, 9, 10, 7, 11, 12, 13]

            pending_out = []

            def emit_out_proj():
                # output projection (c on partitions; host untransposes)
                ob, oxav = pending_out.pop(0)
                for ct in range(CK):
                    for nt in range(NT):
                        ps_o = ps_tile()
                        for idx, t in enumerate(T_ORDER):
                            nc.tensor.matmul(
                                ps_o[:],
                                wo_t[t][:, ct * 128:(ct + 1) * 128],
                                oxav[:, t, nt * 512:(nt + 1) * 512],
                                start=(idx == 0), stop=(idx == TOP - 1))
                        ot = outp.tile([128, 512], BF16, tag="ot", name="ot")
                        nc.scalar.activation(
                            ot[:], ps_o[:],
                            mybir.ActivationFunctionType.Identity,
                            bias=bias_sb[:, ct:ct + 1])
                        nc.sync.dma_start(
                            out_d[ob, ct, :, nt * 512:(nt + 1) * 512], ot[:])

            for b in range(BL):
                xt = xtp.tile([128, CK, N], BF16, tag="xt")

                # ---- DMA issue order: deliver the bytes the first matmul
                # chains need first (xt query-half 0 + wq), then the rest.
                def slab(width):
                    return wp.tile([128, CK, width], BF16, tag="wslab",
                                   name="wslab")

                def load_chunks(t, dram, g):
                    for co in range(CK):
                        nc.sync.dma_start(t[:, co, :], dram[g, :, co, :])

                w_qlo, w_qhi = slab(HG * 128), slab(HG * 96)
                w_vlo, w_vhi = slab(HG * 128), slab(HG * 97)
                for co in range(CK):  # interleave: xt half 0 + wq_lo
                    nc.sync.dma_start(xt[:, co, 0:512],
                                      xt_d[:, co, b * N:b * N + 512])
                    nc.sync.dma_start(w_qlo[:, co, :], wq_lo_d[0, :, co, :])
                load_chunks(w_qhi, wq_hi_d, 0)
                load_chunks(w_vlo, wv_lo_d, 0)
                load_chunks(w_vhi, wv_hi_d, 0)
                for co in range(CK):  # xt half 1
                    nc.sync.dma_start(xt[:, co, 512:1024],
                                      xt_d[:, co, b * N + 512:(b + 1) * N])

                xav_pk = xavp.tile([128, TOP, N], BF16, tag="xav_pk")

                for g in range(GROUPS):
                    # ---- stream this group's projection weights ----
                    if g == 0:
                        w_klo, w_khi = slab(HG * 128), slab(HG * 96)
                        load_chunks(w_klo, wk_lo_d, 0)
                        load_chunks(w_khi, wk_hi_d, 0)
                    else:
                        w_qlo, w_qhi = slab(HG * 128), slab(HG * 96)
                        w_vlo, w_vhi = slab(HG * 128), slab(HG * 97)
                        w_klo, w_khi = slab(HG * 128), slab(HG * 96)
                        load_chunks(w_qlo, wq_lo_d, 1)
                        load_chunks(w_qhi, wq_hi_d, 1)
                        load_chunks(w_vlo, wv_lo_d, 1)
                        load_chunks(w_vhi, wv_hi_d, 1)
                        load_chunks(w_klo, wk_lo_d, 1)
                        load_chunks(w_khi, wk_hi_d, 1)

                    # ---- projections: qT/kT (head-dim on partitions) ----
                    qt_lo = qkvp.tile([128, HG, N], BF16, tag="qt_lo")
                    qt_hi = qkvp.tile([128, HG, N], BF16, tag="qt_hi")
                    kt_lo = qkvp.tile([128, HG, N], BF16, tag="kt_lo")
                    kt_hi = qkvp.tile([128, HG, N], BF16, tag="kt_hi")
                    nc.gpsimd.memset(qt_hi[96:128, :, :], 0.0)
                    nc.gpsimd.memset(kt_hi[96:128, :, :], 0.0)

                    # hi chunks of 4 heads (4x96=384 rows) pack into 3 full
                    # M=128 matmuls; fragments unmixed by the psum->sbuf
                    # copies (pieces respect the 32-aligned partition-window
                    # rule: base 0 may span freely, other bases max 32 rows)
                    HI_FRAGS = {  # ch -> [(hl, src_base, dst_base, rows)]
                        0: [(0, 0, 0, 96), (1, 96, 0, 32)],
                        1: [(1, 0, 32, 32), (1, 32, 64, 32),
                            (2, 64, 0, 32), (2, 96, 32, 32)],
                        2: [(2, 0, 64, 32), (3, 32, 0, 32),
                            (3, 64, 32, 32), (3, 96, 64, 32)],
                    }
                    v_lo = qkvp.tile([128, MC, HG * 128], BF16, tag="v_lo")
                    v_hi = qkvp.tile([128, MC, HG * 97], BF16, tag="v_hi")

                    def qk_proj(wlo, whi, tlo, thi, nt):
                        for hl in range(HG):
                            ps = ps_tile()
                            for co in range(CK):
                                nc.tensor.matmul(
                                    ps[:],
                                    wlo[:, co, hl * 128:(hl + 1) * 128],
                                    xt[:, co, nt * 512:(nt + 1) * 512],
                                    start=(co == 0), stop=(co == CK - 1))
                            nc.vector.tensor_copy(
                                tlo[:, hl, nt * 512:(nt + 1) * 512], ps[:])
                        for ch in range(3):
                            ps = ps_tile()
                            for co in range(CK):
                                nc.tensor.matmul(
                                    ps[:],
                                    whi[:, co, ch * 128:(ch + 1) * 128],
                                    xt[:, co, nt * 512:(nt + 1) * 512],
                                    start=(co == 0), stop=(co == CK - 1))
                            for (hl, sb, db, rows) in HI_FRAGS[ch]:
                                nc.vector.tensor_copy(
                                    thi[db:db + rows, hl,
                                        nt * 512:(nt + 1) * 512],
                                    ps[sb:sb + rows, :])

                    def v_proj(mcs):
                        # natural layout (keys on partitions)
                        for mc in mcs:
                            ps_l = ps_tile()
                            ps_h = ps_tile()
                            for co in range(CK):
                                nc.tensor.matmul(
                                    ps_l[:],
                                    xt[:, co, mc * 128:(mc + 1) * 128],
                                    w_vlo[:, co, :],
                                    start=(co == 0), stop=(co == CK - 1))
                            for co in range(CK):
                                nc.tensor.matmul(
                                    ps_h[:, 0:HG * 97],
                                    xt[:, co, mc * 128:(mc + 1) * 128],
                                    w_vhi[:, co, :],
                                    start=(co == 0), stop=(co == CK - 1))
                            nc.vector.tensor_copy(v_lo[:, mc, :], ps_l[:])
                            nc.vector.tensor_copy(v_hi[:, mc, :],
                                                  ps_h[:, 0:HG * 97])

                    # interleave so early work only needs xt query-half 0
                    # (Q nt0, V keys 0-511, then the half-1 consumers)
                    qk_proj(w_qlo, w_qhi, qt_lo, qt_hi, 0)
                    v_proj(range(0, MC // 2))
                    qk_proj(w_qlo, w_qhi, qt_lo, qt_hi, 1)
                    v_proj(range(MC // 2, MC))
                    qk_proj(w_klo, w_khi, kt_lo, kt_hi, 0)
                    qk_proj(w_klo, w_khi, kt_lo, kt_hi, 1)
                    # ones column per head (softmax denominator row source)
                    for hl in range(HG):
                        nc.gpsimd.memset(v_hi[:, :, hl * 97 + 96], 1.0)

                    # previous batch's output projection: emitted here so the
                    # PE chews dense matmuls while its xav tile is still live
                    if g == 0 and pending_out:
                        emit_out_proj()

                    if g == GROUPS - 1 and not wo_loaded[0]:
                        # prefetch output-projection weights (loaded once,
                        # reused by both batches) under this group's attention
                        for t in range(TOP):
                            nc.sync.dma_start(wo_t[t][:], wo_pk_d[t])
                        wo_loaded[0] = True

                    # ---- attention per head: unnormalized AV into xav_pk ----
                    # Softmax-normalize pipeline: head h's denominator row is
                    # gathered (DMA) as soon as its AV lands, but the recip
                    # (DVE) / broadcast (DMA) / multiplies (GpSimd) emit one
                    # head later, when their inputs have long arrived — the
                    # DVE queue never waits on a DMA round-trip, and the
                    # muls run on the otherwise-idle GpSimd engine so the
                    # next phase's psum->sbuf copies aren't queued behind
                    # them.
                    def norm_tail(h_abs, s_all):
                        rec = xunp.tile([2, 512], BF16, tag="recip")
                        with nc.allow_low_precision(reason="denom recip bf16"):
                            nc.vector.reciprocal(rec[:], s_all[:])
                        r_scr = drp.tile([2, 512], BF16, tag="r_scr")
                        nc.sync.dma_start(r_scr[:], rec[:])
                        for nt in range(NT):
                            nsl = slice(nt * 512, (nt + 1) * 512)
                            bc = outp.tile([128, 512], BF16, tag="bc_sb",
                                           name="bc_sb")
                            nc.sync.dma_start(
                                bc[:],
                                r_scr[nt:nt + 1, :].to_broadcast([128, 512]))
                            nc.vector.tensor_tensor(
                                xav_pk[:, h_abs, nsl],
                                xav_pk[:, h_abs, nsl],
                                bc[:], mybir.AluOpType.mult)
                            for (t, off, rows) in _hi_mul_pieces(h_abs):
                                nc.vector.tensor_tensor(
                                    xav_pk[off:off + rows, t, nsl],
                                    xav_pk[off:off + rows, t, nsl],
                                    bc[off:off + rows, :],
                                    mybir.AluOpType.mult)

                    norm_pending = []
                    for hl in range(HG):
                        h_abs = g * HG + hl
                        for nt in range(NT):
                            expt = expp.tile([128, MC * 512], BF16, tag="expt")
                            for mc in range(MC):
                                ps_sc = ps_tile()
                                nc.tensor.matmul(
                                    ps_sc[:],
                                    kt_lo[:, hl, mc * 128:(mc + 1) * 128],
                                    qt_lo[:, hl, nt * 512:(nt + 1) * 512],
                                    start=True, stop=False)
                                nc.tensor.matmul(
                                    ps_sc[:],
                                    kt_hi[:, hl, mc * 128:(mc + 1) * 128],
                                    qt_hi[:, hl, nt * 512:(nt + 1) * 512],
                                    start=False, stop=True)
                                nc.scalar.activation(
                                    expt[:, mc * 512:(mc + 1) * 512],
                                    ps_sc[:],
                                    mybir.ActivationFunctionType.Exp,
                                    scale=SCALE)
                            ps_alo = ps_av_tile()
                            ps_ahi = ps_av_tile()
                            for mc in range(MC):
                                nc.tensor.matmul(
                                    ps_alo[:],
                                    v_lo[:, mc, hl * 128:(hl + 1) * 128],
                                    expt[:, mc * 512:(mc + 1) * 512],
                                    start=(mc == 0), stop=(mc == MC - 1))
                            for mc in range(MC):
                                nc.tensor.matmul(
                                    ps_ahi[0:97, :],
                                    v_hi[:, mc, hl * 97:(hl + 1) * 97],
                                    expt[:, mc * 512:(mc + 1) * 512],
                                    start=(mc == 0), stop=(mc == MC - 1))
                            # drain PSUM into the packed layout (unnormalized)
                            nsl = slice(nt * 512, (nt + 1) * 512)
                            nc.vector.tensor_copy(
                                xav_pk[:, h_abs, nsl], ps_alo[:])
                            for (j, t, off, rows) in _hi_copy_pieces(h_abs):
                                nc.vector.tensor_copy(
                                    xav_pk[off:off + rows, t, nsl],
                                    ps_ahi[j:j + rows, :])
                            nc.vector.tensor_copy(
                                den2[0:1, h_abs % 2, nsl], ps_ahi[96:97, :])

                        # start this head's denominator gather immediately;
                        # defer its recip/broadcast/muls by one head
                        s_scr = drp.tile([1, N], BF16, tag="s_scr")
                        nc.sync.dma_start(s_scr[:], den2[0:1, h_abs % 2, :])
                        s_all = xunp.tile([2, 512], BF16, tag="s_all")
                        nc.sync.dma_start(
                            s_all[:],
                            s_scr[:].rearrange("o (t i) -> (o t) i", t=NT))
                        if norm_pending:
                            norm_tail(*norm_pending.pop(0))
                        norm_pending.append((h_abs, s_all))
                    while norm_pending:
                        norm_tail(*norm_pending.pop(0))

                pending_out.append((b, xav_pk))

            while pending_out:
                emit_out_proj()

    nc.compile()
    return nc


def _prep_weights(Wq_lr, Wk_lr, Wv_lr, Wout_lr, Wq_full, Wk_full, Wv_full,
                  Wout_full, b_out_full):
    """Host-side weight concat/transpose into device layouts (bf16)."""
    bf16 = ml_dtypes.bfloat16

    def cat_heads(W_lr, W_full):
        # -> (H, 224, C)
        lr = W_lr.reshape(HEADS, RANK, C)
        fl = W_full.reshape(HEADS, DIM_HEAD, C)
        return np.concatenate([lr, fl], axis=1)

    def slab_lo(Wcat):
        # (H,224,C) -> per group [G, 128(p), CK, HG*128] with layout
        # [g][p, co, hl*128+j] = Wcat[g*HG+hl, j, co*128+p]
        A = Wcat[:, :128, :].reshape(GROUPS, HG, 128, CK, 128)
        return np.ascontiguousarray(A.transpose(0, 4, 3, 1, 2)
                                    .reshape(GROUPS, 128, CK, HG * 128)
                                    ).astype(bf16)

    def slab_hi(Wcat, width, pad_to=None):
        A = Wcat[:, 128:224, :].reshape(GROUPS, HG, 96, CK, 128)
        A = A.transpose(0, 4, 3, 1, 2)  # (G, p, co, hl, 96)
        if pad_to is not None:
            pad = np.zeros(A.shape[:-1] + (pad_to - 96,), A.dtype)
            A = np.concatenate([A, pad], axis=-1)
            width = pad_to
        return np.ascontiguousarray(
            A.reshape(GROUPS, 128, CK, HG * width)).astype(bf16)

    Wq_cat = cat_heads(Wq_lr, Wq_full)
    Wk_cat = cat_heads(Wk_lr, Wk_full)
    Wv_cat = cat_heads(Wv_lr, Wv_full)

    # output projection, packed: Wo_cat (H, 224, C), packed rows =
    # [all heads' lo 128s, then all heads' hi 96s] -> (14, 128, C)
    Wo_lr = Wout_lr.reshape(C, HEADS, RANK)
    Wo_fl = Wout_full.reshape(C, HEADS, DIM_HEAD)
    Wo_cat = np.concatenate([Wo_lr, Wo_fl], axis=2).transpose(1, 2, 0)  # (H,224,C)
    wo_pk = np.concatenate(
        [Wo_cat[:, :128, :].reshape(HEADS * 128, C),
         Wo_cat[:, 128:, :].reshape(HEADS * 96, C)], axis=0
    ).reshape(TOP, 128, C).astype(bf16)
    bias_col = np.ascontiguousarray(
        b_out_full.reshape(CK, 128).T).astype(np.float32)

    return {
        "wq_lo": slab_lo(Wq_cat), "wq_hi": slab_hi(Wq_cat, 96),
        "wk_lo": slab_lo(Wk_cat), "wk_hi": slab_hi(Wk_cat, 96),
        "wv_lo": slab_lo(Wv_cat), "wv_hi": slab_hi(Wv_cat, 96, pad_to=97),
        "wo_pk": np.ascontiguousarray(wo_pk), "bias_col": bias_col,
    }


def _prep_xt(hs_core):
    # (BL, N, C) f32 -> [128, CK, BL*N] bf16, xt[p, co, b*N+n] = x[b, n, co*128+p]
    X = hs_core.reshape(BL * N, CK, 128).transpose(2, 1, 0)
    return np.ascontiguousarray(X).astype(ml_dtypes.bfloat16)


_NC_CACHE = {}


def get_nc():
    if "nc" not in _NC_CACHE:
        _NC_CACHE["nc"] = build_bass()
    return _NC_CACHE["nc"]


def kernel(hidden_states, Wq_lr, Wk_lr, Wv_lr, Wout_lr,
           Wq_full, Wk_full, Wv_full, Wout_full, b_out_full):
    hidden_states = np.asarray(hidden_states, np.float32)
    weights = _prep_weights(
        np.asarray(Wq_lr, np.float32), np.asarray(Wk_lr, np.float32),
        np.asarray(Wv_lr, np.float32), np.asarray(Wout_lr, np.float32),
        np.asarray(Wq_full, np.float32), np.asarray(Wk_full, np.float32),
        np.asarray(Wv_full, np.float32), np.asarray(Wout_full, np.float32),
        np.asarray(b_out_full, np.float32))

    in_maps = []
    for c in range(NCORES):
        m = dict(weights)
        m["xt"] = _prep_xt(hidden_states[c * BL:(c + 1) * BL])
        in_maps.append(m)

    nc = get_nc()
    results = run_bass_kernel_spmd(nc, in_maps, core_ids=list(range(NCORES))).results

    out = np.empty((B, N, C), np.float32)
    for c in range(NCORES):
        o = np.asarray(results[c]["out"]).astype(np.float32)  # (BL, CK, 128, N)
        out[c * BL:(c + 1) * BL] = (
            o.transpose(0, 3, 1, 2).reshape(BL, N, C))
    return out


if __name__ == "__main__":
    nc = get_nc()
    print("built + compiled OK")


# revision 18
# speedup vs baseline: 1.0447x; 1.0447x over previous
"""Trainium2 Bass kernel for dual-branch (low-rank + full-rank) self-attention.

Math (per batch b, head h):
  q = x @ Wq_cat[h].T   (N, 224)   224 = 64 (lr) + 160 (full)
  scoresT[m, n] = sum_d K[m, d] Q[n, d]           (keys m on partitions)
  expT = exp(SCALE * scoresT)                     (no max subtraction; f32 psum)
  xav[d, n] = sum_m Vaug[m, d] expT[m, n]         Vaug has a ones column ->
                                                  row 96 of the hi psum = denom
  xnorm = xav * (1/denom)  (per-query recip broadcast via stride-0 DRAM DMA)
  outT[c, n] = sum_t wo_pk[t][:, c-chunk].T @ xav_pk[t]  + bias (activation add)

xav is stored PACKED: 8 heads x 224 dims = 1792 rows = exactly 14 partition
tiles (lo halves of all heads first: tile h = head h dims 0..127; then the
96-row hi parts back to back: row 1024 + h*96 + j). The output projection
contracts 14 full 128-row matmuls per output tile (vs 16 with per-head
128+97 padding) and the bias rides the psum->sbuf copy on the Scalar engine
(activation Identity with a per-partition bias vector).

Softmax denominators are normalized per-head right after each head's AV
matmuls (gather row 96 -> DRAM bounce -> reciprocal -> stride-0 broadcast
-> in-place muls), hidden under the next head's score/AV matmuls.

Sharding: data-parallel, 2 batches per core across 8 cores. No collectives.
All matmuls bf16 with f32 PSUM accumulation; softmax + normalization f32/bf16.
"""

import os
import sys

sys.path.insert(0, "/opt/trn_rl_repo")

import numpy as np
import ml_dtypes

import concourse.bass as bass
import concourse.mybir as mybir
import concourse.tile as tile
from concourse import bacc
from concourse.bass_utils import run_bass_kernel_spmd

# problem constants (hardcoded per spec)
B, N, C = 16, 1024, 1280
HEADS = 8
RANK = 64
DIM_HEAD = 160
DH = RANK + DIM_HEAD          # 224 concat head dim
SCALE = DIM_HEAD ** (-0.5)
NCORES = 8
BL = B // NCORES              # batches per core = 2
CK = C // 128                 # 10 contraction chunks over C
GROUPS = 2                    # head groups per core pass
HG = HEADS // GROUPS          # 4 heads per group
MC = N // 128                 # 8 key chunks
NT = N // 512                 # 2 query-column tiles
TOP = HEADS * DH // 128       # 14 packed xav partition tiles
N_WARM = 18                   # PE warm-up matmuls (HAM clock ungate)

BF16 = mybir.dt.bfloat16
F32 = mybir.dt.float32

# packed hi positions: head h's 96 hi dims start at global row 1024 + h*96
HI_POS = [(8 + (h * 96) // 128, (h * 96) % 128) for h in range(HEADS)]


def _hi_copy_pieces(h):
    """(src_row, dst_tile, dst_off, rows) pieces for the 96-row hi psum->
    packed-xav copy. Fragments respect the 32-aligned partition-window rule:
    base (0,0) may span freely, any other base is limited to 32 rows."""
    t0, off0 = HI_POS[h]
    pieces = []
    j = 0
    while j < 96:
        g = off0 + j
        t = t0 + g // 128
        off = g % 128
        if j == 0 and off == 0:
            n = 96
        else:
            n = min(32, 96 - j, 128 - off)
        pieces.append((j, t, off, n))
        j += n
    return pieces


def _hi_mul_pieces(h):
    """(tile, off, rows) pieces covering head h's hi rows for the in-place
    normalize multiply (base 0 spans freely, other bases 32-row pieces)."""
    t0, off0 = HI_POS[h]
    pieces = []
    j = 0
    while j < 96:
        g = off0 + j
        t = t0 + g // 128
        off = g % 128
        if off == 0:
            n = min(96 - j, 128)
        else:
            n = min(32, 96 - j, 128 - off)
        pieces.append((t, off, n))
        j += n
    return pieces


def build_bass():
    nc = bacc.Bacc("TRN2", target_bir_lowering=False, debug=False,
                   num_devices=NCORES)

    def din(name, shape, dt=BF16):
        return nc.dram_tensor(name, shape, dt, kind="ExternalInput").ap()

    xt_d = din("xt", [128, CK, BL * N])                 # x transposed, c-major
    wq_lo_d = din("wq_lo", [GROUPS, 128, CK, HG * 128])
    wq_hi_d = din("wq_hi", [GROUPS, 128, CK, HG * 96])
    wk_lo_d = din("wk_lo", [GROUPS, 128, CK, HG * 128])
    wk_hi_d = din("wk_hi", [GROUPS, 128, CK, HG * 96])
    wv_lo_d = din("wv_lo", [GROUPS, 128, CK, HG * 128])
    wv_hi_d = din("wv_hi", [GROUPS, 128, CK, HG * 97])  # 97th col zero (ones col)
    wo_pk_d = din("wo_pk", [TOP, 128, C])               # packed output proj
    bias_d = din("bias_col", [128, CK], F32)            # bias, c on partitions
    out_d = nc.dram_tensor("out", [BL, CK, 128, N], BF16,
                           kind="ExternalOutput").ap()

    with tile.TileContext(nc) as tc:
        with (
            tc.tile_pool(name="xtp", bufs=1) as xtp,
            tc.tile_pool(name="wp", bufs=4) as wp,
            tc.tile_pool(name="wop", bufs=TOP) as wop,
            tc.tile_pool(name="qkvp", bufs=1) as qkvp,
            tc.tile_pool(name="xavp", bufs=1) as xavp,
            tc.tile_pool(name="expp", bufs=2) as expp,
            tc.tile_pool(name="cstp", bufs=1) as cstp,
            tc.tile_pool(name="outp", bufs=2) as outp,
            tc.tile_pool(name="xunp", bufs=2) as xunp,
            tc.tile_pool(name="drp", bufs=4, space="DRAM") as drp,
            tc.tile_pool(name="psp", bufs=5, space="PSUM") as psp,
            tc.tile_pool(name="psavp", bufs=3, space="PSUM") as psavp,
        ):
            def ps_tile():
                # general-purpose matmul accumulators (scores/proj/out-proj)
                return psp.tile([128, 512], F32, tag="mm", name="ps")

            def ps_av_tile():
                # AV accumulators — separate slots so the normalize chain
                # never blocks the next head's scores
                return psavp.tile([128, 512], F32, tag="av", name="ps_av")

            # bias column (tiny, load first)
            bias_sb = cstp.tile([128, CK], F32, tag="bias")
            nc.sync.dma_start(bias_sb[:], bias_d)
            # denominator staging row (f32 for reciprocal_approx_fast);
            # nt-half slices rotate between consecutive heads
            den2 = cstp.tile([1, N], F32, tag="denom")

            # PE warm-up: the HAM clock gate needs ~3.4us of sustained
            # activity to ungate 2.4GHz; run zero matmuls while the first
            # xt/weight chunks stream in so real matmuls start warm.
            warm_sb = cstp.tile([128, 640], BF16, tag="warm")
            nc.gpsimd.memset(warm_sb[:], 0.0)
            ps_w = ps_tile()
            for _ in range(N_WARM):
                nc.tensor.matmul(ps_w[:], warm_sb[:, 0:128], warm_sb[:, 128:640],
                                 start=True, stop=True)

            # persistent packed output-projection weights (loaded once)
            wo_t = [wop.tile([128, C], BF16, tag="wo", name="wo")
                    for _ in range(TOP)]
            wo_loaded = [False]
            # out-proj tile order: t7/t13 hold head 7 data (normalized last),
            # so push them late in the accumulation
            T_ORDER = [0, 1, 2, 3, 4, 5, 6, 8, 9, 10, 11, 12, 7, 13]

            pending_out = []

            def emit_out_proj():
                # output projection (c on partitions; host untransposes)
                ob, oxav = pending_out.pop(0)
                for ct in range(CK):
                    for nt in range(NT):
                        ps_o = ps_tile()
                        for idx, t in enumerate(T_ORDER):
                            nc.tensor.matmul(
                                ps_o[:],
                                wo_t[t][:, ct * 128:(ct + 1) * 128],
                                oxav[:, t, nt * 512:(nt + 1) * 512],
                                start=(idx == 0), stop=(idx == TOP - 1))
                        ot = outp.tile([128, 512], BF16, tag="ot", name="ot")
                        nc.scalar.activation(
                            ot[:], ps_o[:],
                            mybir.ActivationFunctionType.Identity,
                            bias=bias_sb[:, ct:ct + 1])
                        nc.sync.dma_start(
                            out_d[ob, ct, :, nt * 512:(nt + 1) * 512], ot[:])

            for b in range(BL):
                xt = xtp.tile([128, CK, N], BF16, tag="xt")

                # ---- DMA issue order: deliver the bytes the first matmul
                # chains need first (xt query-half 0 + wq), then the rest.
                def slab(width):
                    return wp.tile([128, CK, width], BF16, tag="wslab",
                                   name="wslab")

                def load_chunks(t, dram, g):
                    for co in range(CK):
                        nc.sync.dma_start(t[:, co, :], dram[g, :, co, :])

                w_qlo, w_qhi = slab(HG * 128), slab(HG * 96)
                w_vlo, w_vhi = slab(HG * 128), slab(HG * 97)
                for co in range(CK):  # interleave: xt half 0 + wq_lo
                    nc.sync.dma_start(xt[:, co, 0:512],
                                      xt_d[:, co, b * N:b * N + 512])
                    nc.sync.dma_start(w_qlo[:, co, :], wq_lo_d[0, :, co, :])
                load_chunks(w_qhi, wq_hi_d, 0)
                load_chunks(w_vlo, wv_lo_d, 0)
                load_chunks(w_vhi, wv_hi_d, 0)
                for co in range(CK):  # xt half 1
                    nc.sync.dma_start(xt[:, co, 512:1024],
                                      xt_d[:, co, b * N + 512:(b + 1) * N])

                xav_pk = xavp.tile([128, TOP, N], BF16, tag="xav_pk")

                for g in range(GROUPS):
                    # ---- stream this group's projection weights ----
                    if g == 0:
                        w_klo, w_khi = slab(HG * 128), slab(HG * 96)
                        load_chunks(w_klo, wk_lo_d, 0)
                        load_chunks(w_khi, wk_hi_d, 0)
                    else:
                        w_qlo, w_qhi = slab(HG * 128), slab(HG * 96)
                        w_vlo, w_vhi = slab(HG * 128), slab(HG * 97)
                        w_klo, w_khi = slab(HG * 128), slab(HG * 96)
                        load_chunks(w_qlo, wq_lo_d, 1)
                        load_chunks(w_qhi, wq_hi_d, 1)
                        load_chunks(w_vlo, wv_lo_d, 1)
                        load_chunks(w_vhi, wv_hi_d, 1)
                        load_chunks(w_klo, wk_lo_d, 1)
                        load_chunks(w_khi, wk_hi_d, 1)

                    # ---- projections: qT/kT (head-dim on partitions) ----
                    qt_lo = qkvp.tile([128, HG, N], BF16, tag="qt_lo")
                    qt_hi = qkvp.tile([128, HG, N], BF16, tag="qt_hi")
                    kt_lo = qkvp.tile([128, HG, N], BF16, tag="kt_lo")
                    kt_hi = qkvp.tile([128, HG, N], BF16, tag="kt_hi")
                    nc.gpsimd.memset(qt_hi[96:128, :, :], 0.0)
                    nc.gpsimd.memset(kt_hi[96:128, :, :], 0.0)

                    # hi chunks of 4 heads (4x96=384 rows) pack into 3 full
                    # M=128 matmuls; fragments unmixed by the psum->sbuf
                    # copies (pieces respect the 32-aligned partition-window
                    # rule: base 0 may span freely, other bases max 32 rows)
                    HI_FRAGS = {  # ch -> [(hl, src_base, dst_base, rows)]
                        0: [(0, 0, 0, 96), (1, 96, 0, 32)],
                        1: [(1, 0, 32, 32), (1, 32, 64, 32),
                            (2, 64, 0, 32), (2, 96, 32, 32)],
                        2: [(2, 0, 64, 32), (3, 32, 0, 32),
                            (3, 64, 32, 32), (3, 96, 64, 32)],
                    }
                    v_lo = qkvp.tile([128, MC, HG * 128], BF16, tag="v_lo")
                    v_hi = qkvp.tile([128, MC, HG * 97], BF16, tag="v_hi")

                    def qk_proj(wlo, whi, tlo, thi, nt):
                        for hl in range(HG):
                            ps = ps_tile()
                            for co in range(CK):
                                nc.tensor.matmul(
                                    ps[:],
                                    wlo[:, co, hl * 128:(hl + 1) * 128],
                                    xt[:, co, nt * 512:(nt + 1) * 512],
                                    start=(co == 0), stop=(co == CK - 1))
                            nc.vector.tensor_copy(
                                tlo[:, hl, nt * 512:(nt + 1) * 512], ps[:])
                        for ch in range(3):
                            ps = ps_tile()
                            for co in range(CK):
                                nc.tensor.matmul(
                                    ps[:],
                                    whi[:, co, ch * 128:(ch + 1) * 128],
                                    xt[:, co, nt * 512:(nt + 1) * 512],
                                    start=(co == 0), stop=(co == CK - 1))
                            for (hl, sb, db, rows) in HI_FRAGS[ch]:
                                nc.vector.tensor_copy(
                                    thi[db:db + rows, hl,
                                        nt * 512:(nt + 1) * 512],
                                    ps[sb:sb + rows, :])

                    def v_proj(mcs):
                        # natural layout (keys on partitions)
                        for mc in mcs:
                            ps_l = ps_tile()
                            ps_h = ps_tile()
                            for co in range(CK):
                                nc.tensor.matmul(
                                    ps_l[:],
                                    xt[:, co, mc * 128:(mc + 1) * 128],
                                    w_vlo[:, co, :],
                                    start=(co == 0), stop=(co == CK - 1))
                            for co in range(CK):
                                nc.tensor.matmul(
                                    ps_h[:, 0:HG * 97],
                                    xt[:, co, mc * 128:(mc + 1) * 128],
                                    w_vhi[:, co, :],
                                    start=(co == 0), stop=(co == CK - 1))
                            nc.vector.tensor_copy(v_lo[:, mc, :], ps_l[:])
                            nc.vector.tensor_copy(v_hi[:, mc, :],
                                                  ps_h[:, 0:HG * 97])

                    # interleave so early work only needs xt query-half 0
                    # (Q nt0, V keys 0-511, then the half-1 consumers)
                    qk_proj(w_qlo, w_qhi, qt_lo, qt_hi, 0)
                    v_proj(range(0, MC // 2))
                    qk_proj(w_qlo, w_qhi, qt_lo, qt_hi, 1)
                    v_proj(range(MC // 2, MC))
                    qk_proj(w_klo, w_khi, kt_lo, kt_hi, 0)
                    qk_proj(w_klo, w_khi, kt_lo, kt_hi, 1)
                    # ones column per head (softmax denominator row source)
                    for hl in range(HG):
                        nc.gpsimd.memset(v_hi[:, :, hl * 97 + 96], 1.0)

                    # previous batch's output projection: emitted here so the
                    # PE chews dense matmuls while its xav tile is still live
                    if g == 0 and pending_out:
                        emit_out_proj()

                    if g == GROUPS - 1 and not wo_loaded[0]:
                        # prefetch output-projection weights (loaded once,
                        # reused by both batches) under this group's attention
                        for t in range(TOP):
                            nc.sync.dma_start(wo_t[t][:], wo_pk_d[t])
                        wo_loaded[0] = True

                    # ---- attention per head: unnormalized AV into xav_pk ----
                    # Softmax-normalize pipeline: head h's denominator row is
                    # gathered (DMA) as soon as its AV lands, but the recip
                    # (DVE) / broadcast (DMA) / multiplies (GpSimd) emit one
                    # head later, when their inputs have long arrived — the
                    # DVE queue never waits on a DMA round-trip, and the
                    # muls run on the otherwise-idle GpSimd engine so the
                    # next phase's psum->sbuf copies aren't queued behind
                    # them.
                    def norm_tail(h_abs, s_all):
                        rec = xunp.tile([2, 512], F32, tag="recip")
                        nc.vector.reciprocal_approx_fast(rec[:], s_all[:])
                        r_scr = drp.tile([2, 512], F32, tag="r_scr")
                        nc.sync.dma_start(r_scr[:], rec[:])
                        for nt in range(NT):
                            nsl = slice(nt * 512, (nt + 1) * 512)
                            bc = outp.tile([128, 512], F32, tag="bc_sb",
                                           name="bc_sb")
                            nc.sync.dma_start(
                                bc[:],
                                r_scr[nt:nt + 1, :].to_broadcast([128, 512]))
                            nc.vector.tensor_tensor(
                                xav_pk[:, h_abs, nsl],
                                xav_pk[:, h_abs, nsl],
                                bc[:], mybir.AluOpType.mult)
                            for (t, off, rows) in _hi_mul_pieces(h_abs):
                                nc.vector.tensor_tensor(
                                    xav_pk[off:off + rows, t, nsl],
                                    xav_pk[off:off + rows, t, nsl],
                                    bc[off:off + rows, :],
                                    mybir.AluOpType.mult)

                    norm_pending = []
                    for hl in range(HG):
                        h_abs = g * HG + hl
                        for nt in range(NT):
                            expt = expp.tile([128, MC * 512], BF16, tag="expt")
                            for mc in range(MC):
                                ps_sc = ps_tile()
                                nc.tensor.matmul(
                                    ps_sc[:],
                                    kt_lo[:, hl, mc * 128:(mc + 1) * 128],
                                    qt_lo[:, hl, nt * 512:(nt + 1) * 512],
                                    start=True, stop=False)
                                nc.tensor.matmul(
                                    ps_sc[:],
                                    kt_hi[:, hl, mc * 128:(mc + 1) * 128],
                                    qt_hi[:, hl, nt * 512:(nt + 1) * 512],
                                    start=False, stop=True)
                                nc.scalar.activation(
                                    expt[:, mc * 512:(mc + 1) * 512],
                                    ps_sc[:],
                                    mybir.ActivationFunctionType.Exp,
                                    scale=SCALE)
                            ps_alo = ps_av_tile()
                            ps_ahi = ps_av_tile()
                            for mc in range(MC):
                                nc.tensor.matmul(
                                    ps_alo[:],
                                    v_lo[:, mc, hl * 128:(hl + 1) * 128],
                                    expt[:, mc * 512:(mc + 1) * 512],
                                    start=(mc == 0), stop=(mc == MC - 1))
                            for mc in range(MC):
                                nc.tensor.matmul(
                                    ps_ahi[0:97, :],
                                    v_hi[:, mc, hl * 97:(hl + 1) * 97],
                                    expt[:, mc * 512:(mc + 1) * 512],
                                    start=(mc == 0), stop=(mc == MC - 1))
                            # drain PSUM into the packed layout (unnormalized)
                            nsl = slice(nt * 512, (nt + 1) * 512)
                            nc.vector.tensor_copy(
                                xav_pk[:, h_abs, nsl], ps_alo[:])
                            for (j, t, off, rows) in _hi_copy_pieces(h_abs):
                                nc.vector.tensor_copy(
                                    xav_pk[off:off + rows, t, nsl],
                                    ps_ahi[j:j + rows, :])
                            nc.vector.tensor_copy(
                                den2[0:1, nsl], ps_ahi[96:97, :])

                        # start this head's denominator gather immediately;
                        # defer its recip/broadcast/muls by one head
                        s_scr = drp.tile([1, N], F32, tag="s_scr")
                        nc.sync.dma_start(s_scr[:], den2[:])
                        s_all = xunp.tile([2, 512], F32, tag="s_all")
                        nc.sync.dma_start(
                            s_all[:],
                            s_scr[:].rearrange("o (t i) -> (o t) i", t=NT))
                        if norm_pending:
                            norm_tail(*norm_pending.pop(0))
                        norm_pending.append((h_abs, s_all))
                    while norm_pending:
                        norm_tail(*norm_pending.pop(0))

                pending_out.append((b, xav_pk))

            while pending_out:
                emit_out_proj()

    nc.compile()
    return nc


def _prep_weights(Wq_lr, Wk_lr, Wv_lr, Wout_lr, Wq_full, Wk_full, Wv_full,
                  Wout_full, b_out_full):
    """Host-side weight concat/transpose into device layouts (bf16)."""
    bf16 = ml_dtypes.bfloat16

    def cat_heads(W_lr, W_full):
        # -> (H, 224, C)
        lr = W_lr.reshape(HEADS, RANK, C)
        fl = W_full.reshape(HEADS, DIM_HEAD, C)
        return np.concatenate([lr, fl], axis=1)

    def slab_lo(Wcat):
        # (H,224,C) -> per group [G, 128(p), CK, HG*128] with layout
        # [g][p, co, hl*128+j] = Wcat[g*HG+hl, j, co*128+p]
        A = Wcat[:, :128, :].reshape(GROUPS, HG, 128, CK, 128)
        return np.ascontiguousarray(A.transpose(0, 4, 3, 1, 2)
                                    .reshape(GROUPS, 128, CK, HG * 128)
                                    ).astype(bf16)

    def slab_hi(Wcat, width, pad_to=None):
        A = Wcat[:, 128:224, :].reshape(GROUPS, HG, 96, CK, 128)
        A = A.transpose(0, 4, 3, 1, 2)  # (G, p, co, hl, 96)
        if pad_to is not None:
            pad = np.zeros(A.shape[:-1] + (pad_to - 96,), A.dtype)
            A = np.concatenate([A, pad], axis=-1)
            width = pad_to
        return np.ascontiguousarray(
            A.reshape(GROUPS, 128, CK, HG * width)).astype(bf16)

    Wq_cat = cat_heads(Wq_lr, Wq_full)
    Wk_cat = cat_heads(Wk_lr, Wk_full)
    Wv_cat = cat_heads(Wv_lr, Wv_full)

    # output projection, packed: Wo_cat (H, 224, C), packed rows =
    # [all heads' lo 128s, then all heads' hi 96s] -> (14, 128, C)
    Wo_lr = Wout_lr.reshape(C, HEADS, RANK)
    Wo_fl = Wout_full.reshape(C, HEADS, DIM_HEAD)
    Wo_cat = np.concatenate([Wo_lr, Wo_fl], axis=2).transpose(1, 2, 0)  # (H,224,C)
    wo_pk = np.concatenate(
        [Wo_cat[:, :128, :].reshape(HEADS * 128, C),
         Wo_cat[:, 128:, :].reshape(HEADS * 96, C)], axis=0
    ).reshape(TOP, 128, C).astype(bf16)
    bias_col = np.ascontiguousarray(
        b_out_full.reshape(CK, 128).T).astype(np.float32)

    return {
        "wq_lo": slab_lo(Wq_cat), "wq_hi": slab_hi(Wq_cat, 96),
        "wk_lo": slab_lo(Wk_cat), "wk_hi": slab_hi(Wk_cat, 96),
        "wv_lo": slab_lo(Wv_cat), "wv_hi": slab_hi(Wv_cat, 96, pad_to=97),
        "wo_pk": np.ascontiguousarray(wo_pk), "bias_col": bias_col,
    }


def _prep_xt(hs_core):
    # (BL, N, C) f32 -> [128, CK, BL*N] bf16, xt[p, co, b*N+n] = x[b, n, co*128+p]
    X = hs_core.reshape(BL * N, CK, 128).transpose(2, 1, 0)
    return np.ascontiguousarray(X).astype(ml_dtypes.bfloat16)


_NC_CACHE = {}


def get_nc():
    if "nc" not in _NC_CACHE:
        _NC_CACHE["nc"] = build_bass()
    return _NC_CACHE["nc"]


def kernel(hidden_states, Wq_lr, Wk_lr, Wv_lr, Wout_lr,
           Wq_full, Wk_full, Wv_full, Wout_full, b_out_full):
    hidden_states = np.asarray(hidden_states, np.float32)
    weights = _prep_weights(
        np.asarray(Wq_lr, np.float32), np.asarray(Wk_lr, np.float32),
        np.asarray(Wv_lr, np.float32), np.asarray(Wout_lr, np.float32),
        np.asarray(Wq_full, np.float32), np.asarray(Wk_full, np.float32),
        np.asarray(Wv_full, np.float32), np.asarray(Wout_full, np.float32),
        np.asarray(b_out_full, np.float32))

    in_maps = []
    for c in range(NCORES):
        m = dict(weights)
        m["xt"] = _prep_xt(hidden_states[c * BL:(c + 1) * BL])
        in_maps.append(m)

    nc = get_nc()
    results = run_bass_kernel_spmd(nc, in_maps, core_ids=list(range(NCORES))).results

    out = np.empty((B, N, C), np.float32)
    for c in range(NCORES):
        o = np.asarray(results[c]["out"]).astype(np.float32)  # (BL, CK, 128, N)
        out[c * BL:(c + 1) * BL] = (
            o.transpose(0, 3, 1, 2).reshape(BL, N, C))
    return out


if __name__ == "__main__":
    nc = get_nc()
    print("built + compiled OK")


# revision 20
# speedup vs baseline: 1.0615x; 1.0161x over previous
"""Trainium2 Bass kernel for dual-branch (low-rank + full-rank) self-attention.

Math (per batch b, head h):
  q = x @ Wq_cat[h].T   (N, 224)   224 = 64 (lr) + 160 (full)
  scoresT[m, n] = sum_d K[m, d] Q[n, d]           (keys m on partitions)
  expT = exp(SCALE * scoresT)                     (no max subtraction; f32 psum)
  xav[d, n] = sum_m Vaug[m, d] expT[m, n]         Vaug has a ones column ->
                                                  row 96 of the hi psum = denom
  xnorm = xav * (1/denom)  (per-query recip broadcast via stride-0 DRAM DMA)
  outT[c, n] = sum_t wo_pk[t][:, c-chunk].T @ xav_pk[t]  + bias (activation add)

xav is stored PACKED: 8 heads x 224 dims = 1792 rows = exactly 14 partition
tiles (lo halves of all heads first: tile h = head h dims 0..127; then the
96-row hi parts back to back: row 1024 + h*96 + j). The output projection
contracts 14 full 128-row matmuls per output tile (vs 16 with per-head
128+97 padding) and the bias rides the psum->sbuf copy on the Scalar engine
(activation Identity with a per-partition bias vector).

Softmax denominators are normalized per-head right after each head's AV
matmuls (gather row 96 -> DRAM bounce -> reciprocal -> stride-0 broadcast
-> in-place muls), hidden under the next head's score/AV matmuls.

Sharding: data-parallel, 2 batches per core across 8 cores. No collectives.
All matmuls bf16 with f32 PSUM accumulation; softmax + normalization f32/bf16.
"""

import os
import sys

sys.path.insert(0, "/opt/trn_rl_repo")

import numpy as np
import ml_dtypes

import concourse.bass as bass
import concourse.mybir as mybir
import concourse.tile as tile
from concourse import bacc
from concourse.bass_utils import run_bass_kernel_spmd

# problem constants (hardcoded per spec)
B, N, C = 16, 1024, 1280
HEADS = 8
RANK = 64
DIM_HEAD = 160
DH = RANK + DIM_HEAD          # 224 concat head dim
SCALE = DIM_HEAD ** (-0.5)
NCORES = 8
BL = B // NCORES              # batches per core = 2
CK = C // 128                 # 10 contraction chunks over C
GROUPS = 2                    # head groups per core pass
HG = HEADS // GROUPS          # 4 heads per group
MC = N // 128                 # 8 key chunks
NT = N // 512                 # 2 query-column tiles
TOP = HEADS * DH // 128       # 14 packed xav partition tiles
N_WARM = 18                   # PE warm-up matmuls (HAM clock ungate)

BF16 = mybir.dt.bfloat16
F32 = mybir.dt.float32

# packed hi positions: head h's 96 hi dims start at global row 1024 + h*96
HI_POS = [(8 + (h * 96) // 128, (h * 96) % 128) for h in range(HEADS)]


def _hi_copy_pieces(h):
    """(src_row, dst_tile, dst_off, rows) pieces for the 96-row hi psum->
    packed-xav copy. Fragments respect the 32-aligned partition-window rule:
    base (0,0) may span freely, any other base is limited to 32 rows."""
    t0, off0 = HI_POS[h]
    pieces = []
    j = 0
    while j < 96:
        g = off0 + j
        t = t0 + g // 128
        off = g % 128
        if j == 0 and off == 0:
            n = 96
        else:
            n = min(32, 96 - j, 128 - off)
        pieces.append((j, t, off, n))
        j += n
    return pieces


def _hi_mul_pieces(h):
    """(tile, off, rows) pieces covering head h's hi rows for the in-place
    normalize multiply (base 0 spans freely, other bases 32-row pieces)."""
    t0, off0 = HI_POS[h]
    pieces = []
    j = 0
    while j < 96:
        g = off0 + j
        t = t0 + g // 128
        off = g % 128
        if off == 0:
            n = min(96 - j, 128)
        else:
            n = min(32, 96 - j, 128 - off)
        pieces.append((t, off, n))
        j += n
    return pieces


def build_bass():
    nc = bacc.Bacc("TRN2", target_bir_lowering=False, debug=False,
                   num_devices=NCORES)

    def din(name, shape, dt=BF16):
        return nc.dram_tensor(name, shape, dt, kind="ExternalInput").ap()

    xt_d = din("xt", [128, CK, BL * N])                 # x transposed, c-major
    wq_lo_d = din("wq_lo", [GROUPS, 128, CK, HG * 128])
    wq_hi_d = din("wq_hi", [GROUPS, 128, CK, HG * 96])
    wk_lo_d = din("wk_lo", [GROUPS, 128, CK, HG * 128])
    wk_hi_d = din("wk_hi", [GROUPS, 128, CK, HG * 96])
    wv_lo_d = din("wv_lo", [GROUPS, 128, CK, HG * 128])
    wv_hi_d = din("wv_hi", [GROUPS, 128, CK, HG * 97])  # 97th col zero (ones col)
    wo_pk_d = din("wo_pk", [TOP, 128, C])               # packed output proj
    bias_d = din("bias_col", [128, CK], F32)            # bias, c on partitions
    out_d = nc.dram_tensor("out", [BL, CK, 128, N], BF16,
                           kind="ExternalOutput").ap()

    with tile.TileContext(nc) as tc:
        with (
            tc.tile_pool(name="xtp", bufs=1) as xtp,
            tc.tile_pool(name="wp", bufs=4) as wp,
            tc.tile_pool(name="wop", bufs=TOP) as wop,
            tc.tile_pool(name="qkvp", bufs=1) as qkvp,
            tc.tile_pool(name="xavp", bufs=1) as xavp,
            tc.tile_pool(name="expp", bufs=2) as expp,
            tc.tile_pool(name="cstp", bufs=1) as cstp,
            tc.tile_pool(name="outp", bufs=2) as outp,
            tc.tile_pool(name="xunp", bufs=2) as xunp,
            tc.tile_pool(name="drp", bufs=4, space="DRAM") as drp,
            tc.tile_pool(name="psp", bufs=5, space="PSUM") as psp,
            tc.tile_pool(name="psavp", bufs=3, space="PSUM") as psavp,
        ):
            def ps_tile():
                # general-purpose matmul accumulators (scores/proj/out-proj)
                return psp.tile([128, 512], F32, tag="mm", name="ps")

            def ps_av_tile():
                # AV accumulators — separate slots so the normalize chain
                # never blocks the next head's scores
                return psavp.tile([128, 512], F32, tag="av", name="ps_av")

            # bias column (tiny, load first)
            bias_sb = cstp.tile([128, CK], F32, tag="bias")
            nc.sync.dma_start(bias_sb[:], bias_d)
            # denominator staging row (f32 for reciprocal_approx_fast);
            # nt-half slices rotate between consecutive heads
            den2 = cstp.tile([1, N], F32, tag="denom")

            # PE warm-up: the HAM clock gate needs ~3.4us of sustained
            # activity to ungate 2.4GHz; run zero matmuls while the first
            # xt/weight chunks stream in so real matmuls start warm.
            warm_sb = cstp.tile([128, 640], BF16, tag="warm")
            nc.gpsimd.memset(warm_sb[:], 0.0)
            ps_w = ps_tile()
            for _ in range(N_WARM):
                nc.tensor.matmul(ps_w[:], warm_sb[:, 0:128], warm_sb[:, 128:640],
                                 start=True, stop=True)

            # persistent packed output-projection weights (loaded once)
            wo_t = [wop.tile([128, C], BF16, tag="wo", name="wo")
                    for _ in range(TOP)]
            wo_loaded = [False]
            # out-proj tile order: t7/t13 hold head 7 data (normalized last),
            # so push them late in the accumulation
            T_ORDER = [0, 1, 2, 3, 4, 5, 6, 8, 9, 10, 11, 12, 7, 13]

            pending_out = []

            def emit_out_proj():
                # output projection (c on partitions; host untransposes)
                ob, oxav = pending_out.pop(0)
                for nt in range(NT):  # nt-outer: nt1 tiles needed late
                    for ct in range(CK):
                        ps_o = ps_tile()
                        for idx, t in enumerate(T_ORDER):
                            nc.tensor.matmul(
                                ps_o[:],
                                wo_t[t][:, ct * 128:(ct + 1) * 128],
                                oxav[:, t, nt * 512:(nt + 1) * 512],
                                start=(idx == 0), stop=(idx == TOP - 1))
                        ot = outp.tile([128, 512], BF16, tag="ot", name="ot")
                        nc.scalar.activation(
                            ot[:], ps_o[:],
                            mybir.ActivationFunctionType.Identity,
                            bias=bias_sb[:, ct:ct + 1])
                        nc.sync.dma_start(
                            out_d[ob, ct, :, nt * 512:(nt + 1) * 512], ot[:])

            for b in range(BL):
                xt = xtp.tile([128, CK, N], BF16, tag="xt")

                # ---- DMA issue order: deliver the bytes the first matmul
                # chains need first (xt query-half 0 + wq), then the rest.
                def slab(width):
                    return wp.tile([128, CK, width], BF16, tag="wslab",
                                   name="wslab")

                def load_chunks(t, dram, g):
                    for co in range(CK):
                        nc.sync.dma_start(t[:, co, :], dram[g, :, co, :])

                w_qlo, w_qhi = slab(HG * 128), slab(HG * 96)
                w_vlo, w_vhi = slab(HG * 128), slab(HG * 97)
                for co in range(CK):  # interleave: xt half 0 + wq_lo
                    nc.sync.dma_start(xt[:, co, 0:512],
                                      xt_d[:, co, b * N:b * N + 512])
                    nc.sync.dma_start(w_qlo[:, co, :], wq_lo_d[0, :, co, :])
                load_chunks(w_qhi, wq_hi_d, 0)
                load_chunks(w_vlo, wv_lo_d, 0)
                load_chunks(w_vhi, wv_hi_d, 0)
                for co in range(CK):  # xt half 1
                    nc.sync.dma_start(xt[:, co, 512:1024],
                                      xt_d[:, co, b * N + 512:(b + 1) * N])

                xav_pk = xavp.tile([128, TOP, N], BF16, tag="xav_pk")

                for g in range(GROUPS):
                    # ---- stream this group's projection weights ----
                    if g == 0:
                        w_klo, w_khi = slab(HG * 128), slab(HG * 96)
                        load_chunks(w_klo, wk_lo_d, 0)
                        load_chunks(w_khi, wk_hi_d, 0)
                    else:
                        w_qlo, w_qhi = slab(HG * 128), slab(HG * 96)
                        w_vlo, w_vhi = slab(HG * 128), slab(HG * 97)
                        w_klo, w_khi = slab(HG * 128), slab(HG * 96)
                        load_chunks(w_qlo, wq_lo_d, 1)
                        load_chunks(w_qhi, wq_hi_d, 1)
                        load_chunks(w_vlo, wv_lo_d, 1)
                        load_chunks(w_vhi, wv_hi_d, 1)
                        load_chunks(w_klo, wk_lo_d, 1)
                        load_chunks(w_khi, wk_hi_d, 1)

                    # ---- projections: qT/kT (head-dim on partitions) ----
                    qt_lo = qkvp.tile([128, HG, N], BF16, tag="qt_lo")
                    qt_hi = qkvp.tile([128, HG, N], BF16, tag="qt_hi")
                    kt_lo = qkvp.tile([128, HG, N], BF16, tag="kt_lo")
                    kt_hi = qkvp.tile([128, HG, N], BF16, tag="kt_hi")
                    nc.gpsimd.memset(qt_hi[96:128, :, :], 0.0)
                    nc.gpsimd.memset(kt_hi[96:128, :, :], 0.0)

                    # hi chunks of 4 heads (4x96=384 rows) pack into 3 full
                    # M=128 matmuls; fragments unmixed by the psum->sbuf
                    # copies (pieces respect the 32-aligned partition-window
                    # rule: base 0 may span freely, other bases max 32 rows)
                    HI_FRAGS = {  # ch -> [(hl, src_base, dst_base, rows)]
                        0: [(0, 0, 0, 96), (1, 96, 0, 32)],
                        1: [(1, 0, 32, 32), (1, 32, 64, 32),
                            (2, 64, 0, 32), (2, 96, 32, 32)],
                        2: [(2, 0, 64, 32), (3, 32, 0, 32),
                            (3, 64, 32, 32), (3, 96, 64, 32)],
                    }
                    v_lo = qkvp.tile([128, MC, HG * 128], BF16, tag="v_lo")
                    v_hi = qkvp.tile([128, MC, HG * 97], BF16, tag="v_hi")

                    def qk_proj(wlo, whi, tlo, thi, nt):
                        for hl in range(HG):
                            ps = ps_tile()
                            for co in range(CK):
                                nc.tensor.matmul(
                                    ps[:],
                                    wlo[:, co, hl * 128:(hl + 1) * 128],
                                    xt[:, co, nt * 512:(nt + 1) * 512],
                                    start=(co == 0), stop=(co == CK - 1))
                            nc.vector.tensor_copy(
                                tlo[:, hl, nt * 512:(nt + 1) * 512], ps[:])
                        for ch in range(3):
                            ps = ps_tile()
                            for co in range(CK):
                                nc.tensor.matmul(
                                    ps[:],
                                    whi[:, co, ch * 128:(ch + 1) * 128],
                                    xt[:, co, nt * 512:(nt + 1) * 512],
                                    start=(co == 0), stop=(co == CK - 1))
                            for (hl, sb, db, rows) in HI_FRAGS[ch]:
                                nc.vector.tensor_copy(
                                    thi[db:db + rows, hl,
                                        nt * 512:(nt + 1) * 512],
                                    ps[sb:sb + rows, :])

                    def v_proj(mcs):
                        # natural layout (keys on partitions)
                        for mc in mcs:
                            ps_l = ps_tile()
                            ps_h = ps_tile()
                            for co in range(CK):
                                nc.tensor.matmul(
                                    ps_l[:],
                                    xt[:, co, mc * 128:(mc + 1) * 128],
                                    w_vlo[:, co, :],
                                    start=(co == 0), stop=(co == CK - 1))
                            for co in range(CK):
                                nc.tensor.matmul(
                                    ps_h[:, 0:HG * 97],
                                    xt[:, co, mc * 128:(mc + 1) * 128],
                                    w_vhi[:, co, :],
                                    start=(co == 0), stop=(co == CK - 1))
                            nc.vector.tensor_copy(v_lo[:, mc, :], ps_l[:])
                            nc.vector.tensor_copy(v_hi[:, mc, :],
                                                  ps_h[:, 0:HG * 97])

                    # interleave so early work only needs xt query-half 0
                    # (Q nt0, V keys 0-511, then the half-1 consumers)
                    qk_proj(w_qlo, w_qhi, qt_lo, qt_hi, 0)
                    v_proj(range(0, MC // 2))
                    qk_proj(w_qlo, w_qhi, qt_lo, qt_hi, 1)
                    v_proj(range(MC // 2, MC))
                    qk_proj(w_klo, w_khi, kt_lo, kt_hi, 0)
                    qk_proj(w_klo, w_khi, kt_lo, kt_hi, 1)
                    # ones column per head (softmax denominator row source)
                    for hl in range(HG):
                        nc.gpsimd.memset(v_hi[:, :, hl * 97 + 96], 1.0)

                    # previous batch's output projection: emitted here so the
                    # PE chews dense matmuls while its xav tile is still live
                    if g == 0 and pending_out:
                        emit_out_proj()

                    if g == GROUPS - 1 and not wo_loaded[0]:
                        # prefetch output-projection weights (loaded once,
                        # reused by both batches) under this group's attention
                        for t in range(TOP):
                            nc.sync.dma_start(wo_t[t][:], wo_pk_d[t])
                        wo_loaded[0] = True

                    # ---- attention per head: unnormalized AV into xav_pk ----
                    # Softmax-normalize pipeline: head h's denominator row is
                    # gathered (DMA) as soon as its AV lands, but the recip
                    # (DVE) / broadcast (DMA) / multiplies (GpSimd) emit one
                    # head later, when their inputs have long arrived — the
                    # DVE queue never waits on a DMA round-trip, and the
                    # muls run on the otherwise-idle GpSimd engine so the
                    # next phase's psum->sbuf copies aren't queued behind
                    # them.
                    def norm_tail(h_abs, s_all):
                        # muls go to the idle GpSimd engine so they never
                        # delay DVE psum->sbuf copies; except the very last
                        # head, whose muls gate the epilogue out-projection —
                        # DVE is idle and 3x faster there
                        last = (b == BL - 1 and h_abs == HEADS - 1)
                        eng = nc.vector if last else nc.gpsimd
                        rec = xunp.tile([2, 512], F32, tag="recip")
                        nc.vector.reciprocal_approx_fast(rec[:], s_all[:])
                        rec_bf = xunp.tile([2, 512], BF16, tag="recip_bf")
                        nc.vector.tensor_copy(rec_bf[:], rec[:])
                        r_scr = drp.tile([2, 512], BF16, tag="r_scr")
                        nc.sync.dma_start(r_scr[:], rec_bf[:])
                        for nt in range(NT):
                            nsl = slice(nt * 512, (nt + 1) * 512)
                            bc = outp.tile([128, 512], BF16, tag="bc_sb",
                                           name="bc_sb")
                            nc.sync.dma_start(
                                bc[:],
                                r_scr[nt:nt + 1, :].to_broadcast([128, 512]))
                            eng.tensor_tensor(
                                xav_pk[:, h_abs, nsl],
                                xav_pk[:, h_abs, nsl],
                                bc[:], mybir.AluOpType.mult)
                            for (t, off, rows) in _hi_mul_pieces(h_abs):
                                eng.tensor_tensor(
                                    xav_pk[off:off + rows, t, nsl],
                                    xav_pk[off:off + rows, t, nsl],
                                    bc[off:off + rows, :],
                                    mybir.AluOpType.mult)

                    norm_pending = []
                    for hl in range(HG):
                        h_abs = g * HG + hl
                        for nt in range(NT):
                            expt = expp.tile([128, MC * 512], BF16, tag="expt")
                            for mc in range(MC):
                                ps_sc = ps_tile()
                                nc.tensor.matmul(
                                    ps_sc[:],
                                    kt_lo[:, hl, mc * 128:(mc + 1) * 128],
                                    qt_lo[:, hl, nt * 512:(nt + 1) * 512],
                                    start=True, stop=False)
                                nc.tensor.matmul(
                                    ps_sc[:],
                                    kt_hi[:, hl, mc * 128:(mc + 1) * 128],
                                    qt_hi[:, hl, nt * 512:(nt + 1) * 512],
                                    start=False, stop=True)
                                nc.scalar.activation(
                                    expt[:, mc * 512:(mc + 1) * 512],
                                    ps_sc[:],
                                    mybir.ActivationFunctionType.Exp,
                                    scale=SCALE)
                            ps_alo = ps_av_tile()
                            ps_ahi = ps_av_tile()
                            for mc in range(MC):
                                nc.tensor.matmul(
                                    ps_alo[:],
                                    v_lo[:, mc, hl * 128:(hl + 1) * 128],
                                    expt[:, mc * 512:(mc + 1) * 512],
                                    start=(mc == 0), stop=(mc == MC - 1))
                            for mc in range(MC):
                                nc.tensor.matmul(
                                    ps_ahi[0:97, :],
                                    v_hi[:, mc, hl * 97:(hl + 1) * 97],
                                    expt[:, mc * 512:(mc + 1) * 512],
                                    start=(mc == 0), stop=(mc == MC - 1))
                            # drain PSUM into the packed layout (unnormalized)
                            nsl = slice(nt * 512, (nt + 1) * 512)
                            nc.vector.tensor_copy(
                                xav_pk[:, h_abs, nsl], ps_alo[:])
                            for (j, t, off, rows) in _hi_copy_pieces(h_abs):
                                nc.vector.tensor_copy(
                                    xav_pk[off:off + rows, t, nsl],
                                    ps_ahi[j:j + rows, :])
                            nc.vector.tensor_copy(
                                den2[0:1, nsl], ps_ahi[96:97, :])

                        # start this head's denominator gather immediately;
                        # defer its recip/broadcast/muls by one head
                        s_scr = drp.tile([1, N], F32, tag="s_scr")
                        nc.sync.dma_start(s_scr[:], den2[:])
                        s_all = xunp.tile([2, 512], F32, tag="s_all")
                        nc.sync.dma_start(
                            s_all[:],
                            s_scr[:].rearrange("o (t i) -> (o t) i", t=NT))
                        if norm_pending:
                            norm_tail(*norm_pending.pop(0))
                        norm_pending.append((h_abs, s_all))
                    while norm_pending:
                        norm_tail(*norm_pending.pop(0))

                pending_out.append((b, xav_pk))

            while pending_out:
                emit_out_proj()

    nc.compile()
    return nc


def _prep_weights(Wq_lr, Wk_lr, Wv_lr, Wout_lr, Wq_full, Wk_full, Wv_full,
                  Wout_full, b_out_full):
    """Host-side weight concat/transpose into device layouts (bf16)."""
    bf16 = ml_dtypes.bfloat16

    def cat_heads(W_lr, W_full):
        # -> (H, 224, C)
        lr = W_lr.reshape(HEADS, RANK, C)
        fl = W_full.reshape(HEADS, DIM_HEAD, C)
        return np.concatenate([lr, fl], axis=1)

    def slab_lo(Wcat):
        # (H,224,C) -> per group [G, 128(p), CK, HG*128] with layout
        # [g][p, co, hl*128+j] = Wcat[g*HG+hl, j, co*128+p]
        A = Wcat[:, :128, :].reshape(GROUPS, HG, 128, CK, 128)
        return np.ascontiguousarray(A.transpose(0, 4, 3, 1, 2)
                                    .reshape(GROUPS, 128, CK, HG * 128)
                                    ).astype(bf16)

    def slab_hi(Wcat, width, pad_to=None):
        A = Wcat[:, 128:224, :].reshape(GROUPS, HG, 96, CK, 128)
        A = A.transpose(0, 4, 3, 1, 2)  # (G, p, co, hl, 96)
        if pad_to is not None:
            pad = np.zeros(A.shape[:-1] + (pad_to - 96,), A.dtype)
            A = np.concatenate([A, pad], axis=-1)
            width = pad_to
        return np.ascontiguousarray(
            A.reshape(GROUPS, 128, CK, HG * width)).astype(bf16)

    Wq_cat = cat_heads(Wq_lr, Wq_full)
    Wk_cat = cat_heads(Wk_lr, Wk_full)
    Wv_cat = cat_heads(Wv_lr, Wv_full)

    # output projection, packed: Wo_cat (H, 224, C), packed rows =
    # [all heads' lo 128s, then all heads' hi 96s] -> (14, 128, C)
    Wo_lr = Wout_lr.reshape(C, HEADS, RANK)
    Wo_fl = Wout_full.reshape(C, HEADS, DIM_HEAD)
    Wo_cat = np.concatenate([Wo_lr, Wo_fl], axis=2).transpose(1, 2, 0)  # (H,224,C)
    wo_pk = np.concatenate(
        [Wo_cat[:, :128, :].reshape(HEADS * 128, C),
         Wo_cat[:, 128:, :].reshape(HEADS * 96, C)], axis=0
    ).reshape(TOP, 128, C).astype(bf16)
    bias_col = np.ascontiguousarray(
        b_out_full.reshape(CK, 128).T).astype(np.float32)

    return {
        "wq_lo": slab_lo(Wq_cat), "wq_hi": slab_hi(Wq_cat, 96),
        "wk_lo": slab_lo(Wk_cat), "wk_hi": slab_hi(Wk_cat, 96),
        "wv_lo": slab_lo(Wv_cat), "wv_hi": slab_hi(Wv_cat, 96, pad_to=97),
        "wo_pk": np.ascontiguousarray(wo_pk), "bias_col": bias_col,
    }


def _prep_xt(hs_core):
    # (BL, N, C) f32 -> [128, CK, BL*N] bf16, xt[p, co, b*N+n] = x[b, n, co*128+p]
    X = hs_core.reshape(BL * N, CK, 128).transpose(2, 1, 0)
    return np.ascontiguousarray(X).astype(ml_dtypes.bfloat16)


_NC_CACHE = {}


def get_nc():
    if "nc" not in _NC_CACHE:
        _NC_CACHE["nc"] = build_bass()
    return _NC_CACHE["nc"]


def kernel(hidden_states, Wq_lr, Wk_lr, Wv_lr, Wout_lr,
           Wq_full, Wk_full, Wv_full, Wout_full, b_out_full):
    hidden_states = np.asarray(hidden_states, np.float32)
    weights = _prep_weights(
        np.asarray(Wq_lr, np.float32), np.asarray(Wk_lr, np.float32),
        np.asarray(Wv_lr, np.float32), np.asarray(Wout_lr, np.float32),
        np.asarray(Wq_full, np.float32), np.asarray(Wk_full, np.float32),
        np.asarray(Wv_full, np.float32), np.asarray(Wout_full, np.float32),
        np.asarray(b_out_full, np.float32))

    in_maps = []
    for c in range(NCORES):
        m = dict(weights)
        m["xt"] = _prep_xt(hidden_states[c * BL:(c + 1) * BL])
        in_maps.append(m)

    nc = get_nc()
    results = run_bass_kernel_spmd(nc, in_maps, core_ids=list(range(NCORES))).results

    out = np.empty((B, N, C), np.float32)
    for c in range(NCORES):
        o = np.asarray(results[c]["out"]).astype(np.float32)  # (BL, CK, 128, N)
        out[c * BL:(c + 1) * BL] = (
            o.transpose(0, 3, 1, 2).reshape(BL, N, C))
    return out


if __name__ == "__main__":
    nc = get_nc()
    print("built + compiled OK")
